# revision 1
# baseline (speedup 1.0000x reference)
"""IoU / NMS-detection kernel for TRN2 (8 NeuronCores, data-parallel over batch).

Computes, for batch_boxes [32,8732,4] (cxcywh) and batch_gt [32,100,4]:
  ious [32,8732,100] f32, positive_mask = (iou>0.5)&valid, negative_mask = (iou<0.5)&valid

Device strategy (per core, 4 batches):
  - partitions = 128-anchor tiles (N padded 8732->8832 = 69*128), free dim = G=100
  - custom fused DVE ops:
      IOU_DX:    out = relu(min(px2, gx2) - max(px1, gx1))   (bitwise == reference)
      IOU_UNION: out = (area_p + area_g) - inter             (bitwise == reference)
  - inter = dxr*dyr (DVE tt), r = reciprocal_approx_accurate(union) (~2 ULP),
    iou = inter*r, m = ScalarE Sign(iou - 0.5) -> int8 in {-1,0,1}
  - host applies the valid mask to pos/neg (valid is known host-side), since
    invalid gt are made degenerate (coords -1e6, area 0) so iou == 0 exactly.
"""

import os
import numpy as np

import concourse.bacc as bacc
import concourse.mybir as mybir
import concourse.tile as tile
import concourse.dve_ops as dve_ops
from concourse.bass_utils import run_bass_kernel_spmd
from concourse.dve_spec import Spec, Src0, Src1, C0, C1, relu, minn, maxx, lower, _has_src1
from concourse.dve_uop import DveOpSpec

B, N, G = 32, 8732, 100
NCORES = 8
BPC = B // NCORES          # batches per core
NT = 69                    # anchor tiles per batch (padded)
NPAD = NT * 128            # 8832
K = 23                     # tiles per supertile
NST = NT // K              # supertiles per batch

_f32 = mybir.dt.float32
_s8 = mybir.dt.int8


def _register_op(name, spec):
    for op in dve_ops.OPS:
        if op.name == name:
            return op
    row = dve_ops._CUSTOM_DVE_ROW_BASE + len(dve_ops.OPS)
    assert row < 0x20
    dve_ops._SUB_OPCODE_FOR_NAME[name] = row
    sha3 = DveOpSpec(
        name=name, opcode=row, uops=lower(spec, ver="v3"), rd1_en=_has_src1(spec)
    ).sha("v3")
    op = dve_ops.DveOp(name, spec, False, {"v3": sha3})
    dve_ops.OPS.append(op)
    dve_ops.CUSTOM_DVE_SPECS[name] = spec
    return op


IOU_DX = _register_op(
    "IOU_DX_ANT",
    Spec(
        body=relu(minn(C0, Src0) - maxx(C1, Src1)),
        reference=lambda in0, in1, s0, s1, imm2: np.maximum(
            np.minimum(s0, in0.astype(np.float32)) - np.maximum(s1, in1), 0
        ).astype(np.float32),
    ),
)

IOU_UNION = _register_op(
    "IOU_UNION_ANT",
    Spec(
        body=(C0 + Src1) - Src0,
        reference=lambda in0, in1, s0, s1, imm2: (
            (s0 + in1.astype(np.float32)) - in0
        ).astype(np.float32),
    ),
)


_NC_CACHE = {}


def _build_nc():
    nc = bacc.Bacc("TRN2", target_bir_lowering=False, debug=False)
    pf = nc.dram_tensor("pf", [BPC, 128, NT * 5], _f32, kind="ExternalInput")
    gt = nc.dram_tensor("gt", [BPC, 128, 5 * G], _f32, kind="ExternalInput")
    # supertile-major layout: [b, st, p, K*G] -> per-partition contiguous runs
    iou_d = nc.dram_tensor("iou_out", [BPC, NST, 128, K * G], _f32, kind="ExternalOutput")
    m_d = nc.dram_tensor("m_out", [BPC, NST, 128, K * G], _s8, kind="ExternalOutput")

    with tile.TileContext(nc) as tc:
        with tc.tile_pool(name="const", bufs=1) as cpool, tc.tile_pool(
            name="io", bufs=2
        ) as iop, tc.tile_pool(name="st", bufs=2) as stp, tc.tile_pool(
            name="out", bufs=3
        ) as outp:
            neg_half = cpool.tile([128, 1], _f32, tag="neghalf")
            nc.vector.memset(neg_half[:], -0.5)
            for b in range(BPC):
                gt_t = iop.tile([128, 5 * G], _f32, tag="gt")
                pf_t = iop.tile([128, NT * 5], _f32, tag="pf")
                nc.sync.dma_start(out=gt_t[:], in_=gt[b])
                nc.sync.dma_start(out=pf_t[:], in_=pf[b])
                gx1 = gt_t[:, 0:G]
                gx2 = gt_t[:, G : 2 * G]
                gy1 = gt_t[:, 2 * G : 3 * G]
                gy2 = gt_t[:, 3 * G : 4 * G]
                ag = gt_t[:, 4 * G : 5 * G]
                for st in range(NST):
                    dxr = stp.tile([128, K * G], _f32, tag="dxr")
                    dyr = stp.tile([128, K * G], _f32, tag="dyr")
                    inter = stp.tile([128, K * G], _f32, tag="inter")
                    union = stp.tile([128, K * G], _f32, tag="union")
                    r0 = stp.tile([128, K * G], _f32, tag="r0")
                    r1 = stp.tile([128, K * G], _f32, tag="r1")
                    iou = outp.tile([128, K * G], _f32, tag="iou")
                    mm = outp.tile([128, K * G], _s8, tag="mm")
                    for i in range(K):
                        t = st * K + i
                        sl = slice(i * G, (i + 1) * G)
                        px1 = pf_t[:, t * 5 + 0 : t * 5 + 1]
                        px2 = pf_t[:, t * 5 + 1 : t * 5 + 2]
                        py1 = pf_t[:, t * 5 + 2 : t * 5 + 3]
                        py2 = pf_t[:, t * 5 + 3 : t * 5 + 4]
                        nc.vector._custom_dve(
                            IOU_DX, out=dxr[:, sl], in0=gx2, in1=gx1, s0=px2, s1=px1
                        )
                        nc.vector._custom_dve(
                            IOU_DX, out=dyr[:, sl], in0=gy2, in1=gy1, s0=py2, s1=py1
                        )
                    nc.vector.tensor_mul(inter[:], dxr[:], dyr[:])
                    for i in range(K):
                        t = st * K + i
                        sl = slice(i * G, (i + 1) * G)
                        apf = pf_t[:, t * 5 + 4 : t * 5 + 5]
                        nc.vector._custom_dve(
                            IOU_UNION, out=union[:, sl], in0=inter[:, sl], in1=ag, s0=apf
                        )
                    nc.vector.reciprocal_approx_accurate(
                        out=r1[:], in_=union[:], scratch=r0[:]
                    )
                    nc.vector.tensor_mul(iou[:], inter[:], r1[:])
                    nc.scalar.sign(out=mm[:], in_=iou[:], bias=neg_half[:])
                    nsplit = 4
                    step = (K * G) // nsplit  # 575
                    for s in range(nsplit):
                        fsl = slice(s * step, (s + 1) * step)
                        nc.sync.dma_start(
                            out=iou_d[b, st, :, fsl], in_=iou[:, fsl]
                        )
                    nc.sync.dma_start(out=m_d[b, st, :, :], in_=mm[:])
    nc.compile()
    return nc


def _get_nc():
    if "nc" not in _NC_CACHE:
        _NC_CACHE["nc"] = _build_nc()
    return _NC_CACHE["nc"]


def kernel(
    threshhold=None,
    batch_boxes=None,
    batch_classes=None,
    batch_gt=None,
    batch_num_objects=None,
    **_kw,
):
    boxes = np.asarray(batch_boxes, np.float32)
    gtb = np.asarray(batch_gt, np.float32)
    no = np.asarray(batch_num_objects).astype(np.int64)

    half = np.float32(0.5)
    cx, cy, w, h = boxes[..., 0], boxes[..., 1], boxes[..., 2], boxes[..., 3]
    px1 = cx - w * half
    py1 = cy - h * half
    px2 = cx + w * half
    py2 = cy + h * half
    area_p = (px2 - px1) * (py2 - py1)

    def pad(a, fill):
        out = np.full((B, NPAD), fill, np.float32)
        out[:, :N] = a
        return out

    pf = np.stack(
        [pad(px1, -1e4), pad(px2, -1e4), pad(py1, -1e4), pad(py2, -1e4), pad(area_p, 1.0)],
        axis=-1,
    )  # [B, NPAD, 5]
    pf = np.ascontiguousarray(
        pf.reshape(B, NT, 128, 5).transpose(0, 2, 1, 3).reshape(B, 128, NT * 5)
    )

    gcx, gcy, gw, gh = gtb[..., 0], gtb[..., 1], gtb[..., 2], gtb[..., 3]
    gx1 = gcx - gw * half
    gy1 = gcy - gh * half
    gx2 = gcx + gw * half
    gy2 = gcy + gh * half
    area_g = (gx2 - gx1) * (gy2 - gy1)
    validm = np.arange(G)[None, :] < no[:, None]  # [B, G]
    NEG = np.float32(-1e6)
    gx1 = np.where(validm, gx1, NEG).astype(np.float32)
    gx2 = np.where(validm, gx2, NEG).astype(np.float32)
    gy1 = np.where(validm, gy1, NEG).astype(np.float32)
    gy2 = np.where(validm, gy2, NEG).astype(np.float32)
    area_g = np.where(validm, area_g, np.float32(0.0)).astype(np.float32)
    gtpack = np.concatenate([gx1, gx2, gy1, gy2, area_g], axis=1)  # [B, 500]
    gtpack = np.ascontiguousarray(
        np.broadcast_to(gtpack[:, None, :], (B, 128, 5 * G))
    )

    nc = _get_nc()
    in_maps = [
        {
            "pf": np.ascontiguousarray(pf[c * BPC : (c + 1) * BPC]),
            "gt": np.ascontiguousarray(gtpack[c * BPC : (c + 1) * BPC]),
        }
        for c in range(NCORES)
    ]
    trace = os.environ.get("IOU_TRACE", "0") == "1"
    res = run_bass_kernel_spmd(nc, in_maps, list(range(NCORES)), trace=trace)
    _NC_CACHE["last_result"] = res
    results = res.results

    def unscramble(a):
        # [BPC, NST, 128, K*G] -> [BPC, NPAD, G]; anchor n = (st*K+i)*128 + p
        a = a.reshape(BPC, NST, 128, K, G).transpose(0, 1, 3, 2, 4)
        return a.reshape(BPC, NPAD, G)

    iou_full = np.concatenate([unscramble(r["iou_out"]) for r in results], axis=0)
    m_full = np.concatenate([unscramble(r["m_out"]) for r in results], axis=0)
    ious = np.ascontiguousarray(iou_full[:, :N, :])
    m = m_full[:, :N, :]
    vb = validm[:, None, :]
    pos = (m == 1) & vb
    neg = (m == -1) & vb
    return ious, pos, neg



# revision 12
# speedup vs baseline: 1.2580x; 1.2580x over previous
"""IoU / NMS-detection kernel for TRN2 (8 NeuronCores, data-parallel over batch).

Computes, for batch_boxes [32,8732,4] (cxcywh) and batch_gt [32,100,4]:
  ious [32,8732,100] f32, positive_mask = (iou>0.5)&valid, negative_mask = (iou<0.5)&valid

Device strategy (per core, 4 batches):
  - partitions = 128-anchor tiles (N padded 8732->8832 = 69*128), free dim = G=100
  - custom fused DVE ops:
      IOU_DX:    out = relu(min(px2, gx2) - max(px1, gx1))   (bitwise == reference)
      IOU_UNION: out = (area_p + area_g) - inter             (bitwise == reference)
  - inter = dxr*dyr (DVE tt), r = reciprocal_approx_accurate(union) (~2 ULP),
    iou = inter*r, m = ScalarE Sign(iou - 0.5) -> int8 in {-1,0,1}
  - host applies the valid mask to pos/neg (valid is known host-side), since
    invalid gt are made degenerate (coords -1e6, area 0) so iou == 0 exactly.
"""

import os
import numpy as np

import concourse.bacc as bacc
import concourse.mybir as mybir
import concourse.tile as tile
import concourse.dve_ops as dve_ops
from concourse.alu_op_type import AluOpType
from concourse.bass_utils import run_bass_kernel_spmd
from concourse.dve_spec import (
    Spec, Src0, Src1, C0, C1, C2, relu, minn, maxx, lower, _has_src1,
)
from concourse.dve_uop import DveOpSpec

B, N, G = 32, 8732, 100
NCORES = 8
BPC = B // NCORES          # batches per core
NT = 69                    # anchor tiles per batch (padded)
NPAD = NT * 128            # 8832
K = 23                     # tiles per supertile
NST = NT // K              # supertiles per batch

_f32 = mybir.dt.float32
_bf16 = mybir.dt.bfloat16
_s8 = mybir.dt.int8


def _register_op(name, spec):
    for op in dve_ops.OPS:
        if op.name == name:
            return op
    row = dve_ops._CUSTOM_DVE_ROW_BASE + len(dve_ops.OPS)
    assert row < 0x20
    dve_ops._SUB_OPCODE_FOR_NAME[name] = row
    sha3 = DveOpSpec(
        name=name, opcode=row, uops=lower(spec, ver="v3"), rd1_en=_has_src1(spec)
    ).sha("v3")
    op = dve_ops.DveOp(name, spec, False, {"v3": sha3})
    dve_ops.OPS.append(op)
    dve_ops.CUSTOM_DVE_SPECS[name] = spec
    return op


IOU_DX = _register_op(
    "IOU_DX_ANT",
    Spec(
        body=relu(minn(C0, Src0) - maxx(C1, Src1)),
        reference=lambda in0, in1, s0, s1, imm2: np.maximum(
            np.minimum(s0, in0.astype(np.float32)) - np.maximum(s1, in1), 0
        ).astype(np.float32),
    ),
)

# x-direction overlap scaled by C2 (=0.5): dxh = 0.5 * dx. Power-of-2 scale
# is exact in f32, keeping the downstream mask comparison bit-faithful.
IOU_DXS = _register_op(
    "IOU_DXS_ANT",
    Spec(
        body=relu(minn(C0, Src0) - maxx(C1, Src1)) * C2,
        reference=lambda in0, in1, s0, s1, imm2: (
            np.maximum(np.minimum(s0, in0.astype(np.float32)) - np.maximum(s1, in1), 0)
            * imm2
        ).astype(np.float32),
    ),
)

# u4 = (C0 + Src1) - Src0*C2 with C0=area_p/4, Src1=area_g/4, Src0=inter/2,
# C2=0.5  ->  u4 = union/4 (exact power-of-2 scale of the reference union).
IOU_UNION_S = _register_op(
    "IOU_UNION_S_ANT",
    Spec(
        body=(C0 + Src1) - Src0 * C2,
        reference=lambda in0, in1, s0, s1, imm2: (
            (s0 + in1.astype(np.float32)) - in0 * imm2
        ).astype(np.float32),
    ),
)


_NC_CACHE = {}


def _build_nc():
    nc = bacc.Bacc("TRN2", target_bir_lowering=False, debug=False)
    pf = nc.dram_tensor("pf", [BPC, 128, NT * 5], _f32, kind="ExternalInput")
    gt = nc.dram_tensor("gt", [BPC, 128, 5 * G], _f32, kind="ExternalInput")
    # supertile-major layout: [b, st, p, K*G] -> per-partition contiguous runs
    iou_d = nc.dram_tensor("iou_out", [BPC, NST, 128, K * G], _bf16, kind="ExternalOutput")
    m_d = nc.dram_tensor("m_out", [BPC, NST, 128, K * G], _s8, kind="ExternalOutput")

    with tile.TileContext(nc) as tc:
        with tc.tile_pool(name="const", bufs=1) as cpool, tc.tile_pool(
            name="io", bufs=2
        ) as iop, tc.tile_pool(name="st", bufs=2) as stp, tc.tile_pool(
            name="out", bufs=3
        ) as outp:
            zero_b = cpool.tile([128, 1], _f32, tag="zerob")
            nc.vector.memset(zero_b[:], 0.0)
            for b in range(BPC):
                gt_t = iop.tile([128, 5 * G], _f32, tag="gt")
                pf_t = iop.tile([128, NT * 5], _f32, tag="pf")
                nc.sync.dma_start(out=gt_t[:], in_=gt[b])
                nc.sync.dma_start(out=pf_t[:], in_=pf[b])
                gx1 = gt_t[:, 0:G]
                gx2 = gt_t[:, G : 2 * G]
                gy1 = gt_t[:, 2 * G : 3 * G]
                gy2 = gt_t[:, 3 * G : 4 * G]
                ag = gt_t[:, 4 * G : 5 * G]
                for st in range(NST):
                    dxr = stp.tile([128, K * G], _f32, tag="dxr")
                    dyr = stp.tile([128, K * G], _f32, tag="dyr")
                    inter = stp.tile([128, K * G], _f32, tag="inter")
                    union = stp.tile([128, K * G], _f32, tag="union")
                    r1 = stp.tile([128, K * G], _f32, tag="r1")
                    md = stp.tile([128, K * G], _f32, tag="md")
                    iou = outp.tile([128, K * G], _bf16, tag="iou")
                    mm = outp.tile([128, K * G], _s8, tag="mm")
                    for i in range(K):
                        t = st * K + i
                        sl = slice(i * G, (i + 1) * G)
                        px1 = pf_t[:, t * 5 + 0 : t * 5 + 1]
                        px2 = pf_t[:, t * 5 + 1 : t * 5 + 2]
                        py1 = pf_t[:, t * 5 + 2 : t * 5 + 3]
                        py2 = pf_t[:, t * 5 + 3 : t * 5 + 4]
                        nc.vector._custom_dve(
                            IOU_DXS, out=dxr[:, sl], in0=gx2, in1=gx1,
                            s0=px2, s1=px1, imm2=0.5,
                        )
                        nc.vector._custom_dve(
                            IOU_DX, out=dyr[:, sl], in0=gy2, in1=gy1, s0=py2, s1=py1
                        )
                    # inter_h = (dx/2)*dy = inter/2 (exact scale)
                    nc.vector.tensor_mul(inter[:], dxr[:], dyr[:])
                    for i in range(K):
                        t = st * K + i
                        sl = slice(i * G, (i + 1) * G)
                        apf = pf_t[:, t * 5 + 4 : t * 5 + 5]  # area_p/4 (host-packed)
                        nc.vector._custom_dve(
                            IOU_UNION_S, out=union[:, sl], in0=inter[:, sl],
                            in1=ag, s0=apf, imm2=0.5,
                        )
                    # union tile now holds u4 = union/4.
                    # mask: sign(inter/2 - union/4) == sign(2*inter - union)
                    nc.gpsimd.tensor_sub(md[:], inter[:], union[:])
                    nc.scalar.sign(out=mm[:], in_=md[:], bias=zero_b[:])
                    # iou2 = (inter/2) * (4/union) = 2*iou; host halves after upcast
                    nc.vector.reciprocal(out=r1[:], in_=union[:])
                    nc.gpsimd.tensor_mul(iou[:], inter[:], r1[:])
                    nsplit = 2
                    step = (K * G) // nsplit
                    for s in range(nsplit):
                        fsl = slice(s * step, (s + 1) * step)
                        nc.sync.dma_start(
                            out=iou_d[b, st, :, fsl], in_=iou[:, fsl]
                        )
                    nc.sync.dma_start(out=m_d[b, st, :, :], in_=mm[:])
    nc.compile()
    return nc


def _get_nc():
    if "nc" not in _NC_CACHE:
        _NC_CACHE["nc"] = _build_nc()
    return _NC_CACHE["nc"]


def kernel(
    threshhold=None,
    batch_boxes=None,
    batch_classes=None,
    batch_gt=None,
    batch_num_objects=None,
    **_kw,
):
    boxes = np.asarray(batch_boxes, np.float32)
    gtb = np.asarray(batch_gt, np.float32)
    no = np.asarray(batch_num_objects).astype(np.int64)

    half = np.float32(0.5)
    cx, cy, w, h = boxes[..., 0], boxes[..., 1], boxes[..., 2], boxes[..., 3]
    px1 = cx - w * half
    py1 = cy - h * half
    px2 = cx + w * half
    py2 = cy + h * half
    area_p = (px2 - px1) * (py2 - py1)

    def pad(a, fill):
        out = np.full((B, NPAD), fill, np.float32)
        out[:, :N] = a
        return out

    pf = np.stack(
        [pad(px1, -1e4), pad(px2, -1e4), pad(py1, -1e4), pad(py2, -1e4),
         pad(area_p * np.float32(0.25), 0.25)],
        axis=-1,
    )  # [B, NPAD, 5]; area column pre-scaled by 1/4 (exact)
    pf = np.ascontiguousarray(
        pf.reshape(B, NT, 128, 5).transpose(0, 2, 1, 3).reshape(B, 128, NT * 5)
    )

    gcx, gcy, gw, gh = gtb[..., 0], gtb[..., 1], gtb[..., 2], gtb[..., 3]
    gx1 = gcx - gw * half
    gy1 = gcy - gh * half
    gx2 = gcx + gw * half
    gy2 = gcy + gh * half
    area_g = (gx2 - gx1) * (gy2 - gy1)
    validm = np.arange(G)[None, :] < no[:, None]  # [B, G]
    NEG = np.float32(-1e6)
    gx1 = np.where(validm, gx1, NEG).astype(np.float32)
    gx2 = np.where(validm, gx2, NEG).astype(np.float32)
    gy1 = np.where(validm, gy1, NEG).astype(np.float32)
    gy2 = np.where(validm, gy2, NEG).astype(np.float32)
    area_g = np.where(validm, area_g * np.float32(0.25), np.float32(0.0)).astype(
        np.float32
    )
    gtpack = np.concatenate([gx1, gx2, gy1, gy2, area_g], axis=1)  # [B, 500]
    gtpack = np.ascontiguousarray(
        np.broadcast_to(gtpack[:, None, :], (B, 128, 5 * G))
    )

    nc = _get_nc()
    in_maps = [
        {
            "pf": np.ascontiguousarray(pf[c * BPC : (c + 1) * BPC]),
            "gt": np.ascontiguousarray(gtpack[c * BPC : (c + 1) * BPC]),
        }
        for c in range(NCORES)
    ]
    trace = os.environ.get("IOU_TRACE", "0") == "1"
    res = run_bass_kernel_spmd(nc, in_maps, list(range(NCORES)), trace=trace)
    _NC_CACHE["last_result"] = res
    results = res.results

    def unscramble(a):
        # [BPC, NST, 128, K*G] -> [BPC, NPAD, G]; anchor n = (st*K+i)*128 + p
        a = a.reshape(BPC, NST, 128, K, G).transpose(0, 1, 3, 2, 4)
        return a.reshape(BPC, NPAD, G)

    # device emits 2*iou in bf16; halve after upcast (exact in f32)
    iou_full = np.concatenate(
        [unscramble(r["iou_out"].astype(np.float32) * np.float32(0.5)) for r in results],
        axis=0,
    )
    m_full = np.concatenate([unscramble(r["m_out"]) for r in results], axis=0)
    ious = np.ascontiguousarray(iou_full[:, :N, :])
    m = m_full[:, :N, :]
    vb = validm[:, None, :]
    pos = (m == 1) & vb
    neg = (m == -1) & vb
    return ious, pos, neg



# revision 17
# speedup vs baseline: 1.4212x; 1.1297x over previous
"""IoU / NMS-detection kernel for TRN2 (8 NeuronCores, data-parallel over batch).

Computes, for batch_boxes [32,8732,4] (cxcywh) and batch_gt [32,100,4]:
  ious [32,8732,100] f32, positive_mask = (iou>0.5)&valid, negative_mask = (iou<0.5)&valid

Device strategy (per core, 4 batches):
  - partitions = 128-anchor tiles (N padded 8732->8832 = 69*128), free dim = G=100
  - custom fused DVE ops:
      IOU_DX:    out = relu(min(px2, gx2) - max(px1, gx1))   (bitwise == reference)
      IOU_UNION: out = (area_p + area_g) - inter             (bitwise == reference)
  - inter = dxr*dyr (DVE tt), r = reciprocal_approx_accurate(union) (~2 ULP),
    iou = inter*r, m = ScalarE Sign(iou - 0.5) -> int8 in {-1,0,1}
  - host applies the valid mask to pos/neg (valid is known host-side), since
    invalid gt are made degenerate (coords -1e6, area 0) so iou == 0 exactly.
"""

import os
import numpy as np

import concourse.bacc as bacc
import concourse.mybir as mybir
import concourse.tile as tile
import concourse.dve_ops as dve_ops
from concourse.alu_op_type import AluOpType
from concourse.bass_utils import run_bass_kernel_spmd
from concourse.dve_spec import (
    Spec, Src0, Src1, C0, C1, C2, relu, minn, maxx, lower, _has_src1,
)
from concourse.dve_uop import DveOpSpec

B, N, G = 32, 8732, 100
NCORES = 8
BPC = B // NCORES          # batches per core
NT = 69                    # anchor tiles per batch (padded)
NPAD = NT * 128            # 8832
K = 23                     # tiles per supertile
NST = NT // K              # supertiles per batch

_f32 = mybir.dt.float32
_bf16 = mybir.dt.bfloat16
_s8 = mybir.dt.int8
_AFT = mybir.ActivationFunctionType


def _register_op(name, spec):
    for op in dve_ops.OPS:
        if op.name == name:
            return op
    row = dve_ops._CUSTOM_DVE_ROW_BASE + len(dve_ops.OPS)
    assert row < 0x20
    dve_ops._SUB_OPCODE_FOR_NAME[name] = row
    sha3 = DveOpSpec(
        name=name, opcode=row, uops=lower(spec, ver="v3"), rd1_en=_has_src1(spec)
    ).sha("v3")
    op = dve_ops.DveOp(name, spec, False, {"v3": sha3})
    dve_ops.OPS.append(op)
    dve_ops.CUSTOM_DVE_SPECS[name] = spec
    return op


IOU_DX = _register_op(
    "IOU_DX_ANT",
    Spec(
        body=relu(minn(C0, Src0) - maxx(C1, Src1)),
        reference=lambda in0, in1, s0, s1, imm2: np.maximum(
            np.minimum(s0, in0.astype(np.float32)) - np.maximum(s1, in1), 0
        ).astype(np.float32),
    ),
)

# x-direction overlap scaled by C2 (=0.5): dxh = 0.5 * dx. Power-of-2 scale
# is exact in f32, keeping the downstream mask comparison bit-faithful.
IOU_DXS = _register_op(
    "IOU_DXS_ANT",
    Spec(
        body=relu(minn(C0, Src0) - maxx(C1, Src1)) * C2,
        reference=lambda in0, in1, s0, s1, imm2: (
            np.maximum(np.minimum(s0, in0.astype(np.float32)) - np.maximum(s1, in1), 0)
            * imm2
        ).astype(np.float32),
    ),
)

# u4 = (C0 + Src1) - Src0*C2 with C0=area_p/4, Src1=area_g/4, Src0=inter/2,
# C2=0.5  ->  u4 = union/4 (exact power-of-2 scale of the reference union).
IOU_UNION_S = _register_op(
    "IOU_UNION_S_ANT",
    Spec(
        body=(C0 + Src1) - Src0 * C2,
        reference=lambda in0, in1, s0, s1, imm2: (
            (s0 + in1.astype(np.float32)) - in0 * imm2
        ).astype(np.float32),
    ),
)


_NC_CACHE = {}


def _build_nc():
    nc = bacc.Bacc("TRN2", target_bir_lowering=False, debug=False)
    pf = nc.dram_tensor("pf", [BPC, 128, NT * 5], _f32, kind="ExternalInput")
    gt = nc.dram_tensor("gt", [BPC, 128, 5 * G], _f32, kind="ExternalInput")
    # supertile-major layout: [b, st, p, K*G] -> per-partition contiguous runs
    iou_d = nc.dram_tensor("iou_out", [BPC, NST, 128, K * G], _bf16, kind="ExternalOutput")
    m_d = nc.dram_tensor("m_out", [BPC, NST, 128, K * G], _s8, kind="ExternalOutput")

    with tile.TileContext(nc) as tc:
        with tc.tile_pool(name="const", bufs=1) as cpool, tc.tile_pool(
            name="io", bufs=2
        ) as iop, tc.tile_pool(name="st", bufs=2) as stp, tc.tile_pool(
            name="out", bufs=3
        ) as outp:
            # Preload the one act-table set covering Ln+Exp+Sign so the
            # fixpoint loader doesn't thrash tables between Ln and Exp.
            from concourse.hw_specs import get_activation_tables

            tabs = list(get_activation_tables(nc.m.arch).items())
            need = {_AFT.Ln, _AFT.Exp, _AFT.Sign}
            set_id = next(i for i, (_, s) in enumerate(tabs) if need <= s)
            nc.scalar.add_instruction(
                mybir.InstLoadActFuncSet(
                    name=nc.get_next_instruction_name(),
                    act_func_set_id=set_id,
                    engine=mybir.EngineType.Activation,
                    ins=[],
                    outs=[],
                )
            )
            zero_b = cpool.tile([128, 1], _f32, tag="zerob")
            nc.vector.memset(zero_b[:], 0.0)
            for b in range(BPC):
                gt_t = iop.tile([128, 5 * G], _f32, tag="gt")
                pf_t = iop.tile([128, NT * 5], _f32, tag="pf")
                nc.sync.dma_start(out=gt_t[:], in_=gt[b])
                nc.sync.dma_start(out=pf_t[:], in_=pf[b])
                gx1 = gt_t[:, 0:G]
                gx2 = gt_t[:, G : 2 * G]
                gy1 = gt_t[:, 2 * G : 3 * G]
                gy2 = gt_t[:, 3 * G : 4 * G]
                ag = gt_t[:, 4 * G : 5 * G]
                for st in range(NST):
                    dxr = stp.tile([128, K * G], _f32, tag="dxr")
                    dyr = stp.tile([128, K * G], _f32, tag="dyr")
                    inter = stp.tile([128, K * G], _f32, tag="inter")
                    union = stp.tile([128, K * G], _f32, tag="union")
                    r1 = stp.tile([128, K * G], _f32, tag="r1")
                    la = stp.tile([128, K * G], _f32, tag="la")
                    lb = stp.tile([128, K * G], _f32, tag="lb")
                    md = stp.tile([128, K * G], _f32, tag="md")
                    iou = outp.tile([128, K * G], _bf16, tag="iou")
                    mm = outp.tile([128, K * G], _s8, tag="mm")
                    for i in range(K):
                        t = st * K + i
                        sl = slice(i * G, (i + 1) * G)
                        px1 = pf_t[:, t * 5 + 0 : t * 5 + 1]
                        px2 = pf_t[:, t * 5 + 1 : t * 5 + 2]
                        py1 = pf_t[:, t * 5 + 2 : t * 5 + 3]
                        py2 = pf_t[:, t * 5 + 3 : t * 5 + 4]
                        nc.vector._custom_dve(
                            IOU_DXS, out=dxr[:, sl], in0=gx2, in1=gx1,
                            s0=px2, s1=px1, imm2=0.5,
                        )
                        nc.vector._custom_dve(
                            IOU_DX, out=dyr[:, sl], in0=gy2, in1=gy1, s0=py2, s1=py1
                        )
                    # inter_h = (dx/2)*dy = inter/2 (exact scale)
                    nc.vector.tensor_mul(inter[:], dxr[:], dyr[:])
                    for i in range(K):
                        t = st * K + i
                        sl = slice(i * G, (i + 1) * G)
                        apf = pf_t[:, t * 5 + 4 : t * 5 + 5]  # area_p/4 (host-packed)
                        nc.vector._custom_dve(
                            IOU_UNION_S, out=union[:, sl], in0=inter[:, sl],
                            in1=ag, s0=apf, imm2=0.5,
                        )
                    # union tile now holds u4 = union/4.
                    # mask: sign(inter/2 - union/4) == sign(2*inter - union)
                    nc.gpsimd.tensor_sub(md[:], inter[:], union[:])
                    nc.scalar.sign(out=mm[:], in_=md[:], bias=zero_b[:])
                    # iou2 = exp(ln(inter/2) - ln(union/4)) = 2*iou (Act tables;
                    # rel err ~bf16 level for iou >= 1e-15); host halves after upcast
                    nc.scalar.activation(out=la[:], in_=inter[:], func=_AFT.Ln)
                    nc.scalar.activation(out=lb[:], in_=union[:], func=_AFT.Ln)
                    nc.gpsimd.tensor_sub(r1[:], la[:], lb[:])
                    nc.scalar.activation(out=iou[:], in_=r1[:], func=_AFT.Exp)
                    nsplit = 2
                    step = (K * G) // nsplit
                    for s in range(nsplit):
                        fsl = slice(s * step, (s + 1) * step)
                        nc.sync.dma_start(
                            out=iou_d[b, st, :, fsl], in_=iou[:, fsl]
                        )
                    nc.sync.dma_start(out=m_d[b, st, :, :], in_=mm[:])
    nc.compile()
    return nc


def _get_nc():
    if "nc" not in _NC_CACHE:
        _NC_CACHE["nc"] = _build_nc()
    return _NC_CACHE["nc"]


def kernel(
    threshhold=None,
    batch_boxes=None,
    batch_classes=None,
    batch_gt=None,
    batch_num_objects=None,
    **_kw,
):
    boxes = np.asarray(batch_boxes, np.float32)
    gtb = np.asarray(batch_gt, np.float32)
    no = np.asarray(batch_num_objects).astype(np.int64)

    half = np.float32(0.5)
    cx, cy, w, h = boxes[..., 0], boxes[..., 1], boxes[..., 2], boxes[..., 3]
    px1 = cx - w * half
    py1 = cy - h * half
    px2 = cx + w * half
    py2 = cy + h * half
    area_p = (px2 - px1) * (py2 - py1)

    def pad(a, fill):
        out = np.full((B, NPAD), fill, np.float32)
        out[:, :N] = a
        return out

    pf = np.stack(
        [pad(px1, -1e4), pad(px2, -1e4), pad(py1, -1e4), pad(py2, -1e4),
         pad(area_p * np.float32(0.25), 0.25)],
        axis=-1,
    )  # [B, NPAD, 5]; area column pre-scaled by 1/4 (exact)
    pf = np.ascontiguousarray(
        pf.reshape(B, NT, 128, 5).transpose(0, 2, 1, 3).reshape(B, 128, NT * 5)
    )

    gcx, gcy, gw, gh = gtb[..., 0], gtb[..., 1], gtb[..., 2], gtb[..., 3]
    gx1 = gcx - gw * half
    gy1 = gcy - gh * half
    gx2 = gcx + gw * half
    gy2 = gcy + gh * half
    area_g = (gx2 - gx1) * (gy2 - gy1)
    validm = np.arange(G)[None, :] < no[:, None]  # [B, G]
    NEG = np.float32(-1e6)
    gx1 = np.where(validm, gx1, NEG).astype(np.float32)
    gx2 = np.where(validm, gx2, NEG).astype(np.float32)
    gy1 = np.where(validm, gy1, NEG).astype(np.float32)
    gy2 = np.where(validm, gy2, NEG).astype(np.float32)
    area_g = np.where(validm, area_g * np.float32(0.25), np.float32(0.0)).astype(
        np.float32
    )
    gtpack = np.concatenate([gx1, gx2, gy1, gy2, area_g], axis=1)  # [B, 500]
    gtpack = np.ascontiguousarray(
        np.broadcast_to(gtpack[:, None, :], (B, 128, 5 * G))
    )

    nc = _get_nc()
    in_maps = [
        {
            "pf": np.ascontiguousarray(pf[c * BPC : (c + 1) * BPC]),
            "gt": np.ascontiguousarray(gtpack[c * BPC : (c + 1) * BPC]),
        }
        for c in range(NCORES)
    ]
    trace = os.environ.get("IOU_TRACE", "0") == "1"
    res = run_bass_kernel_spmd(nc, in_maps, list(range(NCORES)), trace=trace)
    _NC_CACHE["last_result"] = res
    results = res.results

    def unscramble(a):
        # [BPC, NST, 128, K*G] -> [BPC, NPAD, G]; anchor n = (st*K+i)*128 + p
        a = a.reshape(BPC, NST, 128, K, G).transpose(0, 1, 3, 2, 4)
        return a.reshape(BPC, NPAD, G)

    # device emits 2*iou in bf16; halve after upcast (exact in f32)
    iou_full = np.concatenate(
        [unscramble(r["iou_out"].astype(np.float32) * np.float32(0.5)) for r in results],
        axis=0,
    )
    m_full = np.concatenate([unscramble(r["m_out"]) for r in results], axis=0)
    ious = np.ascontiguousarray(iou_full[:, :N, :])
    m = m_full[:, :N, :]
    vb = validm[:, None, :]
    pos = (m == 1) & vb
    neg = (m == -1) & vb
    return ious, pos, neg



# revision 21
# speedup vs baseline: 1.9190x; 1.3502x over previous
"""IoU / NMS-detection kernel for TRN2 (8 NeuronCores, data-parallel over batch).

Computes, for batch_boxes [32,8732,4] (cxcywh) and batch_gt [32,100,4]:
  ious [32,8732,100] f32, positive_mask = (iou>0.5)&valid, negative_mask = (iou<0.5)&valid

Device strategy (per core, 4 batches):
  - partitions = 128-anchor tiles (N padded 8732->8832 = 69*128), free dim = G=100
  - custom fused DVE ops:
      IOU_DX:    out = relu(min(px2, gx2) - max(px1, gx1))   (bitwise == reference)
      IOU_UNION: out = (area_p + area_g) - inter             (bitwise == reference)
  - inter = dxr*dyr (DVE tt), r = reciprocal_approx_accurate(union) (~2 ULP),
    iou = inter*r, m = ScalarE Sign(iou - 0.5) -> int8 in {-1,0,1}
  - host applies the valid mask to pos/neg (valid is known host-side), since
    invalid gt are made degenerate (coords -1e6, area 0) so iou == 0 exactly.
"""

import os
import numpy as np

import concourse.bacc as bacc
import concourse.mybir as mybir
import concourse.tile as tile
import concourse.dve_ops as dve_ops
from concourse.alu_op_type import AluOpType
from concourse.bass_utils import run_bass_kernel_spmd
from concourse.dve_spec import (
    Spec, Src0, Src1, C0, C1, C2, relu, minn, maxx, lower, _has_src1,
)
from concourse.dve_uop import DveOpSpec

B, N, G = 32, 8732, 100
NCORES = 8
BPC = B // NCORES          # batches per core
NT = 69                    # anchor tiles per batch (padded)
NPAD = NT * 128            # 8832
K = 23                     # tiles per supertile
NST = NT // K              # supertiles per batch

_f32 = mybir.dt.float32
_bf16 = mybir.dt.bfloat16
_s8 = mybir.dt.int8
_AFT = mybir.ActivationFunctionType


def _register_op(name, spec):
    for op in dve_ops.OPS:
        if op.name == name:
            return op
    row = dve_ops._CUSTOM_DVE_ROW_BASE + len(dve_ops.OPS)
    assert row < 0x20
    dve_ops._SUB_OPCODE_FOR_NAME[name] = row
    sha3 = DveOpSpec(
        name=name, opcode=row, uops=lower(spec, ver="v3"), rd1_en=_has_src1(spec)
    ).sha("v3")
    op = dve_ops.DveOp(name, spec, False, {"v3": sha3})
    dve_ops.OPS.append(op)
    dve_ops.CUSTOM_DVE_SPECS[name] = spec
    return op


IOU_DX = _register_op(
    "IOU_DX_ANT",
    Spec(
        body=relu(minn(C0, Src0) - maxx(C1, Src1)),
        reference=lambda in0, in1, s0, s1, imm2: np.maximum(
            np.minimum(s0, in0.astype(np.float32)) - np.maximum(s1, in1), 0
        ).astype(np.float32),
    ),
)

# x-direction overlap scaled by C2 (=0.5): dxh = 0.5 * dx. Power-of-2 scale
# is exact in f32, keeping the downstream mask comparison bit-faithful.
IOU_DXS = _register_op(
    "IOU_DXS_ANT",
    Spec(
        body=relu(minn(C0, Src0) - maxx(C1, Src1)) * C2,
        reference=lambda in0, in1, s0, s1, imm2: (
            np.maximum(np.minimum(s0, in0.astype(np.float32)) - np.maximum(s1, in1), 0)
            * imm2
        ).astype(np.float32),
    ),
)

# u4 = (C0 + Src1) - Src0*C2 with C0=area_p/4, Src1=area_g/4, Src0=inter/2,
# C2=0.5  ->  u4 = union/4 (exact power-of-2 scale of the reference union).
IOU_UNION_S = _register_op(
    "IOU_UNION_S_ANT",
    Spec(
        body=(C0 + Src1) - Src0 * C2,
        reference=lambda in0, in1, s0, s1, imm2: (
            (s0 + in1.astype(np.float32)) - in0 * imm2
        ).astype(np.float32),
    ),
)


_NC_CACHE = {}


def _build_nc(gs):
    """gs: per-batch-slot gt column counts (compile-time), len == BPC."""
    nc = bacc.Bacc("TRN2", target_bir_lowering=False, debug=False)
    pf = nc.dram_tensor("pf", [BPC, 128, NT * 5], _f32, kind="ExternalInput")
    gt_d = [
        nc.dram_tensor(f"gt{b}", [128, 5 * gs[b]], _f32, kind="ExternalInput")
        for b in range(BPC)
    ]
    # supertile-major layout: [st, p, K*Gb] -> per-partition contiguous runs
    iou_d = [
        nc.dram_tensor(f"iou_out{b}", [NST, 128, K * gs[b]], _bf16, kind="ExternalOutput")
        for b in range(BPC)
    ]
    m_d = [
        nc.dram_tensor(f"m_out{b}", [NST, 128, K * gs[b]], _s8, kind="ExternalOutput")
        for b in range(BPC)
    ]

    with tile.TileContext(nc) as tc:
        with tc.tile_pool(name="const", bufs=1) as cpool, tc.tile_pool(
            name="io", bufs=2
        ) as iop, tc.tile_pool(name="st", bufs=2) as stp, tc.tile_pool(
            name="out", bufs=3
        ) as outp:
            # Preload the one act-table set covering Ln+Exp+Sign so the
            # fixpoint loader doesn't thrash tables between Ln and Exp.
            from concourse.hw_specs import get_activation_tables

            tabs = list(get_activation_tables(nc.m.arch).items())
            need = {_AFT.Ln, _AFT.Exp, _AFT.Sign}
            set_id = next(i for i, (_, s) in enumerate(tabs) if need <= s)
            nc.scalar.add_instruction(
                mybir.InstLoadActFuncSet(
                    name=nc.get_next_instruction_name(),
                    act_func_set_id=set_id,
                    engine=mybir.EngineType.Activation,
                    ins=[],
                    outs=[],
                )
            )
            zero_b = cpool.tile([128, 1], _f32, tag="zerob")
            nc.vector.memset(zero_b[:], 0.0)
            for b in range(BPC):
                Gb = gs[b]
                gt_t = iop.tile([128, 5 * Gb], _f32, tag="gt")
                pf_t = iop.tile([128, NT * 5], _f32, tag="pf")
                nc.sync.dma_start(out=gt_t[:], in_=gt_d[b][:, :])
                nc.sync.dma_start(out=pf_t[:], in_=pf[b])
                gx1 = gt_t[:, 0:Gb]
                gx2 = gt_t[:, Gb : 2 * Gb]
                gy1 = gt_t[:, 2 * Gb : 3 * Gb]
                gy2 = gt_t[:, 3 * Gb : 4 * Gb]
                ag = gt_t[:, 4 * Gb : 5 * Gb]
                for st in range(NST):
                    W = K * Gb
                    dxr = stp.tile([128, W], _f32, tag="dxr")
                    dyr = stp.tile([128, W], _f32, tag="dyr")
                    inter = stp.tile([128, W], _f32, tag="inter")
                    union = stp.tile([128, W], _f32, tag="union")
                    r1 = stp.tile([128, W], _f32, tag="r1")
                    la = stp.tile([128, W], _f32, tag="la")
                    lb = stp.tile([128, W], _f32, tag="lb")
                    md = stp.tile([128, W], _f32, tag="md")
                    iou = outp.tile([128, W], _bf16, tag="iou")
                    mm = outp.tile([128, W], _s8, tag="mm")
                    for i in range(K):
                        t = st * K + i
                        sl = slice(i * Gb, (i + 1) * Gb)
                        px1 = pf_t[:, t * 5 + 0 : t * 5 + 1]
                        px2 = pf_t[:, t * 5 + 1 : t * 5 + 2]
                        py1 = pf_t[:, t * 5 + 2 : t * 5 + 3]
                        py2 = pf_t[:, t * 5 + 3 : t * 5 + 4]
                        nc.vector._custom_dve(
                            IOU_DXS, out=dxr[:, sl], in0=gx2, in1=gx1,
                            s0=px2, s1=px1, imm2=0.5,
                        )
                        nc.vector._custom_dve(
                            IOU_DX, out=dyr[:, sl], in0=gy2, in1=gy1, s0=py2, s1=py1
                        )
                    # inter_h = (dx/2)*dy = inter/2 (exact scale)
                    nc.vector.tensor_mul(inter[:], dxr[:], dyr[:])
                    for i in range(K):
                        t = st * K + i
                        sl = slice(i * Gb, (i + 1) * Gb)
                        apf = pf_t[:, t * 5 + 4 : t * 5 + 5]  # area_p/4 (host-packed)
                        nc.vector._custom_dve(
                            IOU_UNION_S, out=union[:, sl], in0=inter[:, sl],
                            in1=ag, s0=apf, imm2=0.5,
                        )
                    # union tile now holds u4 = union/4.
                    # mask: sign(inter/2 - union/4) == sign(2*inter - union)
                    nc.gpsimd.tensor_sub(md[:], inter[:], union[:])
                    nc.scalar.sign(out=mm[:], in_=md[:], bias=zero_b[:])
                    # iou2 = exp(ln(inter/2) - ln(union/4)) = 2*iou (Act tables;
                    # rel err ~bf16 level for iou >= 1e-15); host halves after upcast
                    nc.scalar.activation(out=la[:], in_=inter[:], func=_AFT.Ln)
                    nc.scalar.activation(out=lb[:], in_=union[:], func=_AFT.Ln)
                    nc.gpsimd.tensor_sub(r1[:], la[:], lb[:])
                    nc.scalar.activation(out=iou[:], in_=r1[:], func=_AFT.Exp)
                    nsplit = 2
                    step = W // nsplit
                    for s in range(nsplit):
                        fsl = slice(s * step, (s + 1) * step)
                        nc.sync.dma_start(
                            out=iou_d[b][st, :, fsl], in_=iou[:, fsl]
                        )
                    nc.sync.dma_start(out=m_d[b][st, :, :], in_=mm[:])
    nc.compile()
    return nc


def _get_nc(gs):
    key = tuple(gs)
    if key not in _NC_CACHE:
        _NC_CACHE[key] = _build_nc(key)
    return _NC_CACHE[key]


def kernel(
    threshhold=None,
    batch_boxes=None,
    batch_classes=None,
    batch_gt=None,
    batch_num_objects=None,
    **_kw,
):
    boxes = np.asarray(batch_boxes, np.float32)
    gtb = np.asarray(batch_gt, np.float32)
    no = np.asarray(batch_num_objects).astype(np.int64)

    half = np.float32(0.5)
    cx, cy, w, h = boxes[..., 0], boxes[..., 1], boxes[..., 2], boxes[..., 3]
    px1 = cx - w * half
    py1 = cy - h * half
    px2 = cx + w * half
    py2 = cy + h * half
    area_p = (px2 - px1) * (py2 - py1)

    def pad(a, fill):
        out = np.full((B, NPAD), fill, np.float32)
        out[:, :N] = a
        return out

    pf = np.stack(
        [pad(px1, -1e4), pad(px2, -1e4), pad(py1, -1e4), pad(py2, -1e4),
         pad(area_p * np.float32(0.25), 0.25)],
        axis=-1,
    )  # [B, NPAD, 5]; area column pre-scaled by 1/4 (exact)
    pf = np.ascontiguousarray(
        pf.reshape(B, NT, 128, 5).transpose(0, 2, 1, 3).reshape(B, 128, NT * 5)
    )

    gcx, gcy, gw, gh = gtb[..., 0], gtb[..., 1], gtb[..., 2], gtb[..., 3]
    gx1 = gcx - gw * half
    gy1 = gcy - gh * half
    gx2 = gcx + gw * half
    gy2 = gcy + gh * half
    area_g = (gx2 - gx1) * (gy2 - gy1)
    validm = np.arange(G)[None, :] < no[:, None]  # [B, G]
    NEG = np.float32(-1e6)
    gx1 = np.where(validm, gx1, NEG).astype(np.float32)
    gx2 = np.where(validm, gx2, NEG).astype(np.float32)
    gy1 = np.where(validm, gy1, NEG).astype(np.float32)
    gy2 = np.where(validm, gy2, NEG).astype(np.float32)
    area_g = np.where(validm, area_g * np.float32(0.25), np.float32(0.0)).astype(
        np.float32
    )
    # G-trim: sort batches by num_objects (desc), deal round-robin to cores.
    # Slot s (s-th batch of every core) gets a compile-time gt width
    # Gp[s] = max num_objects in that slot group, padded to a multiple of 4.
    order = np.argsort(-no, kind="stable")  # batch ids, descending num_objects
    gslot = []
    for s in range(BPC):
        grp = order[s * NCORES : (s + 1) * NCORES]
        gmax = int(no[grp].max())
        gslot.append(min(G, -(-gmax // 4) * 4))

    nc = _get_nc(gslot)
    _NC_CACHE["nc"] = nc
    in_maps = []
    for c in range(NCORES):
        m_in = {"pf": np.ascontiguousarray(pf[[order[s * NCORES + c] for s in range(BPC)]])}
        for s in range(BPC):
            b = order[s * NCORES + c]
            Gs = gslot[s]
            gtp = np.concatenate(
                [gx1[b, :Gs], gx2[b, :Gs], gy1[b, :Gs], gy2[b, :Gs], area_g[b, :Gs]]
            )  # [5*Gs]
            m_in[f"gt{s}"] = np.ascontiguousarray(
                np.broadcast_to(gtp[None, :], (128, 5 * Gs))
            )
        in_maps.append(m_in)
    trace = os.environ.get("IOU_TRACE", "0") == "1"
    res = run_bass_kernel_spmd(nc, in_maps, list(range(NCORES)), trace=trace)
    _NC_CACHE["last_result"] = res
    results = res.results

    def unscramble(a, Gs):
        # [NST, 128, K*Gs] -> [NPAD, Gs]; anchor n = (st*K+i)*128 + p
        a = a.reshape(NST, 128, K, Gs).transpose(0, 2, 1, 3)
        return a.reshape(NPAD, Gs)

    ious = np.zeros((B, N, G), np.float32)
    m = np.zeros((B, N, G), np.int8)
    for c in range(NCORES):
        r = results[c]
        for s in range(BPC):
            b = int(order[s * NCORES + c])
            Gs = gslot[s]
            # device emits 2*iou in bf16; halve after upcast (exact in f32)
            iou_b = unscramble(
                r[f"iou_out{s}"].astype(np.float32) * np.float32(0.5), Gs
            )
            ious[b, :, :Gs] = iou_b[:N]
            m[b, :, :Gs] = unscramble(r[f"m_out{s}"], Gs)[:N]
    vb = validm[:, None, :]
    pos = (m == 1) & vb
    neg = (m == -1) & vb
    return ious, pos, neg

